# revision 21
# baseline (speedup 1.0000x reference)
"""Trainium2 Bass kernel for nn_CLOSpreadModel (bucketed hinge ensemble).

out = hinge(mvoc; base) + bucket_adj(mvoc, bucket_idx) + hinge(lev_idx; idx)
    + hinge(wap) + hinge(cpnspread; cpn) + hinge(equity_nav; nav) + bias
with hinge(x) = b + sum_k w_k * relu(x - t_k).

Fixed hinges (5 x 32 knots), two routes per knot:
  "PE" route: ACT computes relu(w*x - w*t) (one pass per knot, bf16) and the
    PE accumulates it into PSUM via +/-identity matmuls (sign of w baked).
    For w<0 the identity w*relu(x-t) = w*(x-t) - |w|*relu(t-x) folds an
    affine correction into per-feature constants.
  "A2" route (exact f32, on DVE): y = w*x - w*t;  term = max(y,0) (w>0) or
    min(y,0) (w<0);  accumulated into an SBUF accumulator.

Bucket adjustment (16 tiny hinges, row-routed by bucket_idx):
  Processed in a DMA-transposed layout: rows go along the free dim, 4 natural
  partitions ("quarters") stacked as 4x32 partition groups. One-hot bucket
  masks (is_equal against per-partition bucket ids, two copies for hi/lo bf16
  coefficient splits) are mixed by the PE into per-row knot vectors:
     delta_k = x - t[b,k]   (PSUM, f32-exact via bf16 hi/lo splits)
     omega_k = w[b,k]
  ACT applies relu (PSUM->SBUF bf16), DVE multiplies by omega, and a second
  PE contraction sums over the 32 knot slots (plus the per-bucket bias via
  the mask tile) directly into the fixed-hinge PSUM accumulator.
"""
import numpy as np
from contextlib import ExitStack

import concourse.bass as bass
import concourse.mybir as mybir
from concourse.bass_utils import run_bass_kernel_spmd

ALU = mybir.AluOpType
DT = mybir.dt
AF = mybir.ActivationFunctionType

N = 2_097_152
NCORES = 8
NC_ROWS = N // NCORES          # 262144
P = 128
F = NC_ROWS // P               # 2048 free dim
HF = F // 2
HB = F // 4                    # bucket piece columns
K = 32
B = 16
NKNOT = 5 * K
NSLOT = 8
NCHK = 32                      # bucket chunks (4 natural partitions each)
N_A2 = 64                      # knots routed to the exact DVE path
X_NAMES = ["mvoc", "lev", "wap", "cpn", "nav"]
KNOT_FEAT = [fi for fi in range(5) for _ in range(K)]


def _build_program(signs, route):
    nc = bass.Bass(detect_race_conditions=False)
    xin = {n: nc.declare_dram_parameter(n, [P, F], DT.float32, isOutput=False)
           for n in X_NAMES}
    ab_in = nc.declare_dram_parameter("ab", [P, 2 * NKNOT], DT.float32, isOutput=False)
    aff_in = nc.declare_dram_parameter("aff", [P, 8], DT.float32, isOutput=False)
    ident_in = nc.declare_dram_parameter("ident", [P, P], DT.bfloat16, isOutput=False)
    nident_in = nc.declare_dram_parameter("nident", [P, P], DT.bfloat16, isOutput=False)
    bidx_in = nc.declare_dram_parameter("bidx", [P, F], DT.int32, isOutput=False)
    wdel_in = nc.declare_dram_parameter("wdel", [P, P], DT.bfloat16, isOutput=False)
    womg_in = nc.declare_dram_parameter("womg", [P, P], DT.bfloat16, isOutput=False)
    wx_in = nc.declare_dram_parameter("wx", [8, P], DT.bfloat16, isOutput=False)
    wp2_in = nc.declare_dram_parameter("wp2", [P, 8], DT.bfloat16, isOutput=False)
    bvec_in = nc.declare_dram_parameter("bvec", [P, 2], DT.float32, isOutput=False)
    y_out = nc.declare_dram_parameter("y", [P, F], DT.float32, isOutput=True)

    act_knots = [gk for gk in range(NKNOT) if route[gk] == "PE"]
    a2_knots = [gk for gk in range(NKNOT) if route[gk] == "A2"]
    n_act = len(act_knots)
    n_a2 = len(a2_knots)

    with ExitStack() as ctx:
        ec = ctx.enter_context
        x = {n: ec(nc.sbuf_tensor(f"x_{n}", [P, F], DT.float32)) for n in X_NAMES}
        ab = ec(nc.sbuf_tensor("ab_t", [P, 2 * NKNOT], DT.float32))
        aff_p = ec(nc.sbuf_tensor("aff_p", [P, 8], DT.float32))
        ident = ec(nc.sbuf_tensor("ident_t", [P, P], DT.bfloat16))
        nident = ec(nc.sbuf_tensor("nident_t", [P, P], DT.bfloat16))
        u = [ec(nc.sbuf_tensor(f"u{i}", [P, F], DT.bfloat16)) for i in range(NSLOT)]
        aff = ec(nc.sbuf_tensor("aff_t", [P, F], DT.float32))
        out_t = ec(nc.sbuf_tensor("out_t", [P, F], DT.float32))
        bidx = ec(nc.sbuf_tensor("bidx_t", [P, F], DT.int32))
        bidx_f = ec(nc.sbuf_tensor("bidx_f", [P, F], DT.float32))
        x_hi = ec(nc.sbuf_tensor("x_hi", [P, F], DT.bfloat16))
        x_lo = ec(nc.sbuf_tensor("x_lo", [P, F], DT.bfloat16))
        wdel = ec(nc.sbuf_tensor("wdel_t", [P, P], DT.bfloat16))
        womg = ec(nc.sbuf_tensor("womg_t", [P, P], DT.bfloat16))
        wx = ec(nc.sbuf_tensor("wx_t", [8, P], DT.bfloat16))
        wp2 = ec(nc.sbuf_tensor("wp2_t", [P, 8], DT.bfloat16))
        bvec = ec(nc.sbuf_tensor("bvec_t", [P, 2], DT.float32))
        ua2 = ec(nc.sbuf_tensor("ua2_t", [P, F], DT.float32))
        dacc = ec(nc.sbuf_tensor("dacc_t", [P, F], DT.float32))
        brep = [ec(nc.sbuf_tensor(f"brep{i}", [P, F], DT.float32)) for i in range(2)]
        mask = [ec(nc.sbuf_tensor(f"mask{i}", [P, F], DT.bfloat16)) for i in range(2)]
        xT = [ec(nc.sbuf_tensor(f"xT{i}", [8, F], DT.bfloat16)) for i in range(2)]
        reld = [ec(nc.sbuf_tensor(f"reld{i}", [P, HB], DT.bfloat16)) for i in range(2)]
        vt = [ec(nc.sbuf_tensor(f"vt{i}", [P, HB], DT.bfloat16)) for i in range(2)]
        adj_sb = [ec(nc.sbuf_tensor(f"adj_sb{i}", [4, 2 * HB], DT.float32)) for i in range(4)]
        adj_nat = ec(nc.sbuf_tensor("adj_nat", [P, F], DT.float32))
        psum = ec(nc.psum_tensor("ps_acc", [P, F], DT.float32))
        ps_d = ec(nc.psum_tensor("ps_d", [P, HB], DT.float32))
        ps_o = ec(nc.psum_tensor("ps_o", [P, HB], DT.float32))
        ps_a = ec(nc.psum_tensor("ps_a", [4, 2 * HB], DT.float32))
        dma_sem = ec(nc.semaphore())
        dma2_sem = ec(nc.semaphore())
        ready_sem = ec(nc.semaphore())
        prep_sem = ec(nc.semaphore())
        act_sem = ec(nc.semaphore())
        peid_sem = ec(nc.semaphore())
        mask_sem = ec(nc.semaphore())
        pe1_sem = ec(nc.semaphore())
        reld_sem = ec(nc.semaphore())
        v_sem = ec(nc.semaphore())
        pe2_sem = ec(nc.semaphore())
        aesc_sem = ec(nc.semaphore())
        dma3_sem = ec(nc.semaphore())
        chunk_rdy = ec(nc.semaphore())
        resh_rdy = ec(nc.semaphore())
        out_sem = ec(nc.semaphore())
        block = ec(nc.Block())

        @block.gpsimd
        def _(g):
            for n in X_NAMES:
                g.dma_start(out=x[n][:], in_=xin[n][:]).then_inc(dma_sem, 16)
            for dst, srcp in [(ab, ab_in), (aff_p, aff_in), (ident, ident_in),
                              (nident, nident_in), (bidx, bidx_in), (wdel, wdel_in),
                              (womg, womg_in), (wx, wx_in), (wp2, wp2_in),
                              (bvec, bvec_in)]:
                g.dma_start(out=dst[:], in_=srcp[:]).then_inc(dma_sem, 16)
            g.wait_ge(dma_sem, 16 * 15)
            g.sem_inc(ready_sem, 1)
            g.wait_ge(prep_sem, 1)
            for c in range(NCHK):
                if c >= 2:
                    g.wait_ge(mask_sem, c - 1)
                    g.wait_ge(pe1_sem, 12 * (c - 2) + 11)
                sl = c % 2
                for q in range(4):
                    g.dma_start(
                        out=brep[sl][32 * q:32 * q + 32, :],
                        in_=bidx_f[4 * c + q:4 * c + q + 1, :]
                            .rearrange("q (o f) -> q o f", o=1)
                            .broadcast_to((1, 32, F)),
                    ).then_inc(dma2_sem, 16)
                g.dma_start(out=xT[sl][0:4, :], in_=x_hi[4 * c:4 * c + 4, :])\
                    .then_inc(dma2_sem, 16)
                g.dma_start(out=xT[sl][4:8, :], in_=x_lo[4 * c:4 * c + 4, :])\
                    .then_inc(dma2_sem, 16)
                g.wait_ge(dma2_sem, 96 * (c + 1))
                g.sem_inc(chunk_rdy, 1)
                if c >= 3:
                    for bt in (2 * (c - 3), 2 * (c - 3) + 1):
                        g.wait_ge(aesc_sem, bt + 1)
                        for i in range(2):
                            t_ = 2 * bt + i
                            c2, pc = t_ // 4, t_ % 4
                            g.dma_start(
                                out=adj_nat[4 * c2:4 * c2 + 4, HB * pc:HB * pc + HB],
                                in_=adj_sb[bt % 4][:, HB * i:HB * i + HB],
                            ).then_inc(dma3_sem, 16)
                        g.wait_ge(dma3_sem, 32 * (bt + 1))
                        g.sem_inc(resh_rdy, 1)
            for bt in range(2 * (NCHK - 3), 2 * NCHK):
                g.wait_ge(aesc_sem, bt + 1)
                for i in range(2):
                    t_ = 2 * bt + i
                    c2, pc = t_ // 4, t_ % 4
                    g.dma_start(
                        out=adj_nat[4 * c2:4 * c2 + 4, HB * pc:HB * pc + HB],
                        in_=adj_sb[bt % 4][:, HB * i:HB * i + HB],
                    ).then_inc(dma3_sem, 16)
                g.wait_ge(dma3_sem, 32 * (bt + 1))
                g.sem_inc(resh_rdy, 1)
            g.wait_ge(out_sem, 1)
            g.dma_start(out=y_out[:], in_=out_t[:]).then_inc(dma_sem, 16)
            g.wait_ge(dma_sem, 16 * 16)

        @block.scalar
        def _(s):
            s.wait_ge(ready_sem, 1)
            ai = 0
            for c in range(NCHK):
                upto = ((c + 1) * n_act) // NCHK
                while ai < upto:
                    gk = act_knots[ai]
                    if ai >= NSLOT and ai % 4 == 0:
                        s.wait_ge(peid_sem, 4 * (ai - (NSLOT - 1)))
                    xn = X_NAMES[KNOT_FEAT[gk]]
                    nc.scalar.activation(
                        out=u[ai % NSLOT][:], in_=x[xn][:], func=AF.Relu,
                        scale=ab[:, 2 * gk:2 * gk + 1],
                        bias=ab[:, 2 * gk + 1:2 * gk + 2],
                    ).then_inc(act_sem, 1)
                    ai += 1
                for pc in range(4):
                    t_ = 4 * c + pc
                    if t_ >= 2:
                        s.wait_ge(v_sem, t_ - 1)     # reld slot free
                    s.wait_ge(pe1_sem, 12 * c + 3 * pc + 2)  # ps_d ready
                    nc.scalar.activation(
                        out=reld[t_ % 2][:], in_=ps_d[:], func=AF.Relu,
                        scale=1.0, bias=bvec[:, 1:2],
                    ).then_inc(reld_sem, 1)
                    if t_ % 2 == 1 and t_ >= 3:
                        bt = (t_ - 3) // 2
                        if bt >= 4:
                            s.wait_ge(resh_rdy, bt - 3)  # adj_sb slot free
                        s.wait_ge(pe2_sem, 2 * (2 * bt + 1) + 2)
                        nc.scalar.copy(out=adj_sb[bt % 4][:], in_=ps_a[:])\
                            .then_inc(aesc_sem, 1)
            bt = 2 * NCHK - 1
            s.wait_ge(resh_rdy, bt - 3)
            s.wait_ge(pe2_sem, 2 * (2 * bt + 1) + 2)
            nc.scalar.copy(out=adj_sb[bt % 4][:], in_=ps_a[:]).then_inc(aesc_sem, 1)

        @block.tensor
        def _(t):
            t.wait_ge(ready_sem, 1)
            state = {"ai": 0}

            def id_accums(upto):
                while state["ai"] < upto:
                    ai = state["ai"]
                    gk = act_knots[ai]
                    t.wait_ge(act_sem, ai + 1)
                    lt = ident if signs[gk] > 0 else nident
                    for bk in range(4):
                        nc.tensor.matmul(
                            out=psum[:, 512 * bk:512 * (bk + 1)],
                            lhsT=lt[:],
                            rhs=u[ai % NSLOT][:, 512 * bk:512 * (bk + 1)],
                            start=(ai == 0), stop=(ai == n_act - 1),
                            skip_group_check=True,
                        ).then_inc(peid_sem, 1)
                    state["ai"] += 1

            id_accums(min(1, n_act))
            for c in range(NCHK):
                sl = c % 2
                t.wait_ge(mask_sem, c + 1)
                t.wait_ge(chunk_rdy, c + 1)
                def pe2(t2):
                    c2, pc2 = t2 // 4, t2 % 4
                    blk = t2 % 2
                    if t2 >= 2:
                        t.wait_ge(aesc_sem, (t2 - 2) // 2 + 1)  # ps_a blk escaped
                    t.wait_ge(v_sem, t2 + 1)
                    nc.tensor.matmul(
                        out=ps_a[:, HB * blk:HB * blk + HB],
                        lhsT=wp2[:, 0:4], rhs=vt[t2 % 2][:],
                        start=True, stop=False, skip_group_check=True,
                    ).then_inc(pe2_sem, 1)
                    nc.tensor.matmul(
                        out=ps_a[:, HB * blk:HB * blk + HB],
                        lhsT=wp2[:, 4:8],
                        rhs=mask[(c2 % 2)][:, HB * pc2:HB * pc2 + HB],
                        start=False, stop=True, skip_group_check=True,
                    ).then_inc(pe2_sem, 1)

                for pc in range(4):
                    t_ = 4 * c + pc
                    if t_ >= 1:
                        t.wait_ge(reld_sem, t_)      # ps_d free
                    nc.tensor.matmul(
                        out=ps_d[:], lhsT=wdel[:], rhs=mask[sl][:, HB * pc:HB * pc + HB],
                        start=True, stop=False, skip_group_check=True,
                    ).then_inc(pe1_sem, 1)
                    nc.tensor.matmul(
                        out=ps_d[:], lhsT=wx[:], rhs=xT[sl][:, HB * pc:HB * pc + HB],
                        start=False, stop=True, skip_group_check=True,
                    ).then_inc(pe1_sem, 1)
                    if t_ >= 1:
                        t.wait_ge(v_sem, t_)         # ps_o free
                    nc.tensor.matmul(
                        out=ps_o[:], lhsT=womg[:], rhs=mask[sl][:, HB * pc:HB * pc + HB],
                        start=True, stop=True, skip_group_check=True,
                    ).then_inc(pe1_sem, 1)
                    if t_ >= 2:
                        pe2(t_ - 2)
                id_accums(((c + 1) * n_act) // NCHK)
            pe2(4 * NCHK - 2)
            pe2(4 * NCHK - 1)

        @block.vector
        def _(v):
            v.wait_ge(ready_sem, 1)
            nc.vector.tensor_copy(out=bidx_f[:], in_=bidx[:])
            nc.vector.tensor_copy(out=x_hi[:], in_=x["mvoc"][:])
            nc.vector.tensor_tensor(out=x_lo[:], in0=x["mvoc"][:], in1=x_hi[:],
                                    op=ALU.subtract)
            v.sem_inc(prep_sem, 1)
            nc.vector.tensor_scalar(
                out=aff[:], in0=x["mvoc"][:], scalar1=aff_p[:, 0:1],
                scalar2=aff_p[:, 5:6], op0=ALU.mult, op1=ALU.add)
            for fi in range(1, 5):
                nc.vector.scalar_tensor_tensor(
                    out=aff[:], in0=x[X_NAMES[fi]][:], scalar=aff_p[:, fi:fi + 1],
                    in1=aff[:], op0=ALU.mult, op1=ALU.add)
            a2i = 0
            first_a2 = True
            for c in range(NCHK):
                v.wait_ge(chunk_rdy, c + 1)
                if c >= 2:
                    v.wait_ge(pe2_sem, 8 * (c - 2) + 8)   # mask slot free
                nc.vector.tensor_scalar(
                    out=mask[c % 2][:], in0=brep[c % 2][:], scalar1=bvec[:, 0:1],
                    scalar2=None, op0=ALU.is_equal).then_inc(mask_sem, 1)
                upto = ((c + 1) * n_a2) // NCHK
                while a2i < upto:
                    gk = a2_knots[a2i]
                    xn = X_NAMES[KNOT_FEAT[gk]]
                    clip_op = ALU.max if signs[gk] > 0 else ALU.min
                    nc.vector.tensor_scalar(
                        out=ua2[:], in0=x[xn][:], scalar1=ab[:, 2 * gk:2 * gk + 1],
                        scalar2=ab[:, 2 * gk + 1:2 * gk + 2], op0=ALU.mult, op1=ALU.add)
                    if first_a2:
                        nc.vector.tensor_scalar(
                            out=dacc[:], in0=ua2[:], scalar1=0.0, scalar2=None,
                            op0=clip_op)
                        first_a2 = False
                    else:
                        nc.vector.scalar_tensor_tensor(
                            out=dacc[:], in0=ua2[:], scalar=0.0, op0=clip_op,
                            in1=dacc[:], op1=ALU.add)
                    a2i += 1
                for pc in range(4):
                    t_ = 4 * c + pc
                    if t_ >= 2:
                        v.wait_ge(pe2_sem, 2 * (t_ - 2) + 1)  # vt slot free
                    v.wait_ge(reld_sem, t_ + 1)
                    v.wait_ge(pe1_sem, 12 * c + 3 * pc + 3)   # ps_o ready
                    nc.vector.scalar_tensor_tensor(
                        out=vt[t_ % 2][:], in0=reld[t_ % 2][:], scalar=0.0,
                        op0=ALU.add, in1=ps_o[:], op1=ALU.mult).then_inc(v_sem, 1)
            if n_a2 > 0:
                nc.vector.tensor_tensor(out=aff[:], in0=aff[:], in1=dacc[:], op=ALU.add)
            v.wait_ge(resh_rdy, 2 * NCHK)
            nc.vector.tensor_tensor(out=aff[:], in0=aff[:], in1=adj_nat[:], op=ALU.add)
            v.wait_ge(peid_sem, 4 * n_act)
            nc.vector.tensor_tensor(out=out_t[:], in0=aff[:], in1=psum[:],
                                    op=ALU.add).then_inc(out_sem, 1)

    return nc


_CACHE = {}


def _get_program(signs, route):
    key = (tuple(signs), tuple(route))
    if key not in _CACHE:
        _CACHE[key] = _build_program(key[0], key[1])
    return _CACHE[key]


def _bf16(a):
    import ml_dtypes
    return np.asarray(a, np.float32).astype(ml_dtypes.bfloat16)


def _bf16_split(a):
    """f32 -> (hi, lo) bf16 pair with hi+lo ~= a to ~2^-17 relative."""
    import ml_dtypes
    a = np.asarray(a, np.float32)
    hi = a.astype(ml_dtypes.bfloat16)
    lo = (a - hi.astype(np.float32)).astype(ml_dtypes.bfloat16)
    return hi, lo


def _host_params(inp):
    hs = [(inp["base_knots"], inp["base_w"]), (inp["idx_knots"], inp["idx_w"]),
          (inp["wap_knots"], inp["wap_w"]), (inp["cpn_knots"], inp["cpn_w"]),
          (inp["nav_knots"], inp["nav_w"])]
    tb = np.concatenate([np.asarray(t, np.float64) for t, _ in hs])
    wb = np.concatenate([np.asarray(w, np.float64) for _, w in hs])
    # route: exact DVE path for the knots with the largest relu magnitudes
    mag = np.abs(wb) * (5.5 - tb)
    a2_set = set(np.argsort(-mag)[:N_A2].tolist())
    route = tuple("A2" if gk in a2_set else "PE" for gk in range(NKNOT))
    signs = tuple(1 if wb[gk] >= 0 else -1 for gk in range(NKNOT))

    ab = np.zeros(2 * NKNOT, np.float64)
    A = np.zeros(5, np.float64)
    B_total = (float(inp["base_b"]) + float(inp["idx_b"]) + float(inp["wap_b"])
               + float(inp["cpn_b"]) + float(inp["nav_b"]) + float(inp["bias"]))
    for gk in range(NKNOT):
        w, tk = wb[gk], tb[gk]
        ab[2 * gk] = w
        ab[2 * gk + 1] = -w * tk
        if w < 0 and route[gk] == "PE":
            A[KNOT_FEAT[gk]] += w
            B_total += -w * tk
    return ab.astype(np.float32), A.astype(np.float32), np.float32(B_total), \
        signs, route


def _bucket_weights(adj_knots, adj_w, adj_b):
    """Build wdel/womg [128,128], wx [8,128], wp2 [128,8] bf16 tiles."""
    import ml_dtypes
    t_hi, t_lo = _bf16_split(-np.asarray(adj_knots, np.float32))   # [B, K]
    w_hi, w_lo = _bf16_split(np.asarray(adj_w, np.float32))
    a_hi, a_lo = _bf16_split(np.asarray(adj_b, np.float32))        # [B]
    wdel = np.zeros((P, P), ml_dtypes.bfloat16)
    womg = np.zeros((P, P), ml_dtypes.bfloat16)
    wx = np.zeros((8, P), ml_dtypes.bfloat16)
    wp2 = np.zeros((P, 8), ml_dtypes.bfloat16)
    for p in range(P):
        q, i = p // 32, p % 32          # mask partition p = q*32 + i
        b = i % 16
        hi = i < 16
        for k in range(K):
            m = q * 32 + k
            wdel[p, m] = t_hi[b, k] if hi else t_lo[b, k]
            womg[p, m] = w_hi[b, k] if hi else w_lo[b, k]
        wp2[p, 4 + q] = a_hi[b] if hi else a_lo[b]
    for row in range(8):
        q = row % 4
        for k in range(K):
            wx[row, q * 32 + k] = np.float32(1.0)
    for m in range(P):
        q = m // 32
        wp2[m, q] = np.float32(1.0)
    return wdel, womg, wx, wp2


def kernel(**inputs):
    inp = {k: np.asarray(v) for k, v in inputs.items()}
    ab, A, B_total, signs, route = _host_params(inp)
    wdel, womg, wx, wp2 = _bucket_weights(inp["adj_knots"], inp["adj_w"],
                                          inp["adj_b"])
    ab_tile = np.broadcast_to(ab, (P, 2 * NKNOT)).copy()
    aff_tile = np.zeros((P, 8), np.float32)
    aff_tile[:, 0:5] = A
    aff_tile[:, 5] = B_total
    bvec_tile = np.zeros((P, 2), np.float32)
    bvec_tile[:, 0] = [(p % 32) % 16 for p in range(P)]
    ident_bf16 = _bf16(np.eye(P, dtype=np.float32))
    nident_bf16 = _bf16(-np.eye(P, dtype=np.float32))

    feats = {"mvoc": inp["mvoc"], "lev": inp["lev_idx"], "wap": inp["wap"],
             "cpn": inp["cpnspread"], "nav": inp["equity_nav"]}
    bidx_full = inp["bucket_idx"].reshape(-1).astype(np.int32)
    in_maps = []
    for c in range(NCORES):
        sl = slice(c * NC_ROWS, (c + 1) * NC_ROWS)
        m = {n: np.ascontiguousarray(feats[n][sl].reshape(P, F)) for n in feats}
        m["bidx"] = np.ascontiguousarray(bidx_full[sl].reshape(P, F))
        m["ab"] = ab_tile
        m["aff"] = aff_tile
        m["ident"] = ident_bf16
        m["nident"] = nident_bf16
        m["wdel"] = wdel
        m["womg"] = womg
        m["wx"] = wx
        m["wp2"] = wp2
        m["bvec"] = bvec_tile
        in_maps.append(m)

    nc = _get_program(signs, route)
    res = run_bass_kernel_spmd(nc, in_maps, list(range(NCORES)))
    out = np.empty((N,), np.float32)
    for c in range(NCORES):
        out[c * NC_ROWS:(c + 1) * NC_ROWS] = res.results[c]["y"].reshape(-1)
    return out
